# revision 7
# baseline (speedup 1.0000x reference)
"""Trainium2 Bass kernel for DecoderAttention (B=4, S=2048, H=1024, NH=16).

Sharding: 8 cores = (batch b in 0..4) x (head-group hg in 0..2, 8 heads each).
Each core:
  - projects q/k/v for its 8 heads from x[b]      (weights pre-transposed on host)
  - causal attention, softmax without max-subtraction (scores bounded ~ +-3)
  - writes its [8, 2048, 2048] slice of `attn`
  - computes a partial o-projection (its 512 columns of the head dim)
Host sums the two partials per batch and adds the bias terms
(bo + bv @ Wo.T folds out of the device kernel because attn rows sum to 1).
"""

import numpy as np


def _import_concourse():
    try:
        import concourse.bass  # noqa: F401
    except ImportError:
        import sys

        for p in ("/root/.axon_site/_ro/trn_rl_repo", "/opt/trn_rl_repo"):
            if p not in sys.path:
                sys.path.append(p)


_import_concourse()

import concourse.bass as bass  # noqa: E402
import concourse.mybir as mybir  # noqa: E402
import concourse.tile as tile  # noqa: E402
from concourse import bacc  # noqa: E402
from concourse.masks import make_causal_mask, make_identity  # noqa: E402

B, S, H, NH = 4, 2048, 1024, 16
HD = H // NH  # 64
SCALE = float(HD) ** -0.5
NCORES = 8
C = H // 2  # 512 columns of the concat-head dim per core (8 heads)
F32 = mybir.dt.float32
F32R = mybir.dt.float32r
F16 = mybir.dt.float16
MASK_VAL = -1e10  # additive pre-softmax mask; exp(SCALE*(s+MASK_VAL)) == 0 in fp32

# Use float32r (full-rate fp32 matmul mode) for the big matmuls. The BIR
# verifier requires every fp32r-matmul operand to be *produced* as fp32r,
# so the feeding tiles (and their DRAM sources) are declared float32r.
MM_F32R = True
MM_DT = F32R if MM_F32R else F32


def _mm(ap):
    return ap


def _build_kernel_body(tc):
    nc = tc.nc
    from contextlib import ExitStack

    xT = nc.dram_tensor("xT", [H, S], MM_DT, kind="ExternalInput").ap()
    wqT = nc.dram_tensor("wqT", [H, C], MM_DT, kind="ExternalInput").ap()
    wkT = nc.dram_tensor("wkT", [H, C], MM_DT, kind="ExternalInput").ap()
    wvT = nc.dram_tensor("wvT", [H, C], MM_DT, kind="ExternalInput").ap()
    woT = nc.dram_tensor("woT", [C, H], F16, kind="ExternalInput").ap()
    bq = nc.dram_tensor("bq", [C], F32, kind="ExternalInput").ap()
    bk = nc.dram_tensor("bk", [C], F32, kind="ExternalInput").ap()
    attn = nc.dram_tensor("attn", [NH // 2, S, S], F32, kind="ExternalOutput").ap()
    out_part = nc.dram_tensor("out_part", [S, H], F32, kind="ExternalOutput").ap()

    HPC = NH // 2  # 8 heads per core
    NK = H // 128  # 8 contraction tiles for the projections
    NM = C // 128  # 4 o-tiles per projection
    NST = S // 128  # 16 s-tiles
    NSC = S // 512  # 4 s-chunks
    NQB = S // 128  # 16 query blocks

    with ExitStack() as ctx:
        consts = ctx.enter_context(tc.tile_pool(name="consts", bufs=1))
        identity_h = consts.tile([128, 128], F16)
        make_identity(nc, identity_h)
        cmask = consts.tile([128, 128], F32)
        make_causal_mask(nc, cmask, mask_val=MASK_VAL)

        qkpool = ctx.enter_context(tc.tile_pool(name="qk", bufs=2 * NM))
        vpool = ctx.enter_context(tc.tile_pool(name="v", bufs=NST))
        qT_sb = [qkpool.tile([128, S], MM_DT, tag="qk", name=f"qT{m}") for m in range(NM)]
        kT_sb = [qkpool.tile([128, S], MM_DT, tag="qk", name=f"kT{m}") for m in range(NM)]
        v_sb = [vpool.tile([128, C], F16, tag="v", name=f"v{s}") for s in range(NST)]

        ps_mm = ctx.enter_context(tc.tile_pool(name="ps_mm", bufs=2, space="PSUM"))

        # ---------- Phase B: q/k/v projections ----------
        with ExitStack() as bctx:
            xpool = bctx.enter_context(tc.tile_pool(name="x", bufs=NK))
            wtile = bctx.enter_context(tc.tile_pool(name="wt", bufs=2 * NK))
            wvpool = bctx.enter_context(tc.tile_pool(name="wv", bufs=NK))
            biasp = bctx.enter_context(tc.tile_pool(name="bias", bufs=2 * NM))

            xT_sb = [xpool.tile([128, S], MM_DT, tag="x", name=f"xT{k}") for k in range(NK)]
            for k in range(NK):
                nc.sync.dma_start(out=xT_sb[k], in_=xT[128 * k : 128 * (k + 1), :])

            bq_sb = [biasp.tile([128, 1], F32, tag="bias", name=f"bqs{m}") for m in range(NM)]
            bk_sb = [biasp.tile([128, 1], F32, tag="bias", name=f"bks{m}") for m in range(NM)]
            for m in range(NM):
                nc.sync.dma_start(
                    out=bq_sb[m], in_=bq[128 * m : 128 * (m + 1)].rearrange("(p one) -> p one", one=1)
                )
                nc.sync.dma_start(
                    out=bk_sb[m], in_=bk[128 * m : 128 * (m + 1)].rearrange("(p one) -> p one", one=1)
                )

            # qT / kT: out[o, s] accumulated over k; lhsT = w[k_tile, o_tile]
            for dst, wsrc, bias_sb in ((qT_sb, wqT, bq_sb), (kT_sb, wkT, bk_sb)):
                for m in range(NM):
                    wk_tiles = []
                    for k in range(NK):
                        wt = wtile.tile([128, 128], MM_DT, tag="wt", name=f"w{m}_{k}")
                        nc.sync.dma_start(
                            out=wt,
                            in_=wsrc[128 * k : 128 * (k + 1), 128 * m : 128 * (m + 1)],
                        )
                        wk_tiles.append(wt)
                    for sc in range(NSC):
                        ps = ps_mm.tile([128, 512], F32, tag="ps_mm")
                        for k in range(NK):
                            nc.tensor.matmul(
                                ps,
                                _mm(wk_tiles[k]),
                                _mm(xT_sb[k][:, 512 * sc : 512 * (sc + 1)]),
                                start=(k == 0),
                                stop=(k == NK - 1),
                            )
                        nc.scalar.activation(
                            dst[m][:, 512 * sc : 512 * (sc + 1)],
                            ps,
                            mybir.ActivationFunctionType.Identity,
                            bias=bias_sb[m],
                            scale=1.0,
                        )

            # v: out[s, c] (natural layout); lhsT = xT tile, rhs = wvT tile
            wv_tiles = []
            for k in range(NK):
                wt = wvpool.tile([128, C], MM_DT, tag="wv", name=f"wv{k}")
                nc.sync.dma_start(out=wt, in_=wvT[128 * k : 128 * (k + 1), :])
                wv_tiles.append(wt)
            for st in range(NST):
                ps = ps_mm.tile([128, 512], F32, tag="ps_mm")
                for k in range(NK):
                    nc.tensor.matmul(
                        ps,
                        _mm(xT_sb[k][:, 128 * st : 128 * (st + 1)]),
                        _mm(wv_tiles[k]),
                        start=(k == 0),
                        stop=(k == NK - 1),
                    )
                nc.vector.tensor_copy(v_sb[st], ps)

        # ---------- Phase C: attention ----------
        aopool = ctx.enter_context(tc.tile_pool(name="ao", bufs=NM))
        attnoutT = [aopool.tile([128, S], F16, tag="ao", name=f"ao{m}") for m in range(NM)]

        zpool = ctx.enter_context(tc.tile_pool(name="z", bufs=1))
        zero_t = zpool.tile([128, S], F32)
        nc.gpsimd.memset(zero_t, 0.0)

        ppool = ctx.enter_context(tc.tile_pool(name="p", bufs=4))
        phpool = ctx.enter_context(tc.tile_pool(name="ph", bufs=4))
        ptpool = ctx.enter_context(tc.tile_pool(name="pt", bufs=3))
        dpool = ctx.enter_context(tc.tile_pool(name="d", bufs=16))
        ps_pt = ctx.enter_context(tc.tile_pool(name="ps_pt", bufs=2, space="PSUM"))
        ps_av = ctx.enter_context(tc.tile_pool(name="ps_av", bufs=2, space="PSUM"))

        for h in range(HPC):
            mt = h // 2  # which 128-partition qT/kT tile
            po = 64 * (h % 2)  # partition offset of this head inside the tile
            qTh = qT_sb[mt][po : po + 64, :]
            kTh = kT_sb[mt][po : po + 64, :]
            for i in range(NQB // 2):
                p_pair = []
                kepair = 128 * (2 * i + 2)
                for qb in (2 * i, 2 * i + 1):
                    ke = 128 * (qb + 1)  # causal key extent for this q block
                    P = ppool.tile([128, S], F32, tag="p")
                    Ph = phpool.tile([128, S], F16, tag="ph")
                    p_pair.append(Ph)
                    nchunks = (ke + 511) // 512
                    dparts = []
                    for c in range(nchunks):
                        w = min(512, ke - 512 * c)
                        ps = ps_mm.tile([128, 512], F32, tag="ps_mm")
                        nc.tensor.matmul(
                            ps[:, :w],
                            _mm(qTh[:, 128 * qb : 128 * (qb + 1)]),
                            _mm(kTh[:, 512 * c : 512 * c + w]),
                            start=True,
                            stop=True,
                        )
                        if 512 * c <= ke - 128 < 512 * c + w:
                            off = (ke - 128) - 512 * c
                            nc.vector.tensor_add(
                                ps[:, off : off + 128], ps[:, off : off + 128], cmask
                            )
                        d = dpool.tile([128, 1], F32, tag="d")
                        nc.scalar.activation(
                            P[:, 512 * c : 512 * c + w],
                            ps[:, :w],
                            mybir.ActivationFunctionType.Exp,
                            scale=SCALE,
                            accum_out=d,
                        )
                        dparts.append(d)
                    dsum = dparts[0]
                    for d in dparts[1:]:
                        dn = dpool.tile([128, 1], F32, tag="d")
                        nc.vector.tensor_add(dn, dsum, d)
                        dsum = dn
                    invd = dpool.tile([128, 1], F32, tag="d")
                    nc.vector.reciprocal(invd, dsum)
                    # normalized fp16 shadow for the attn@v transposes; must
                    # trace BEFORE the in-place fp32 normalization below so it
                    # reads the raw exp output
                    nc.gpsimd.tensor_scalar_mul(Ph[:, :ke], P[:, :ke], invd)
                    if qb % 2 == 0:
                        # the pair's attn@v runs to the odd block's extent
                        nc.gpsimd.memset(Ph[:, ke:kepair], 0.0)
                    nc.vector.tensor_scalar_mul(P[:, :ke], P[:, :ke], invd)
                    nc.sync.dma_start(
                        out=attn[h, 128 * qb : 128 * (qb + 1), :ke], in_=P[:, :ke]
                    )
                    if ke < S:
                        nc.sync.dma_start(
                            out=attn[h, 128 * qb : 128 * (qb + 1), ke:],
                            in_=zero_t[:, : S - ke],
                        )

                kt = 2 * i + 2  # key tiles covered by this pair
                av = ps_av.tile([64, 256], F32, tag="ps_av")
                for j in range(kt):
                    ptp = ps_pt.tile([128, 256], F16, tag="ps_pt")
                    nc.tensor.transpose(
                        ptp[:, 0:128], p_pair[0][:, 128 * j : 128 * (j + 1)], identity_h
                    )
                    nc.tensor.transpose(
                        ptp[:, 128:256], p_pair[1][:, 128 * j : 128 * (j + 1)], identity_h
                    )
                    pt = ptpool.tile([128, 256], F16, tag="pt")
                    if j % 2 == 0:
                        nc.scalar.copy(pt, ptp)
                    else:
                        nc.vector.tensor_copy(pt, ptp)
                    nc.tensor.matmul(
                        av,
                        _mm(v_sb[j][:, 64 * h : 64 * h + 64]),
                        _mm(pt),
                        start=(j == 0),
                        stop=(j == kt - 1),
                    )
                nc.scalar.copy(
                    attnoutT[mt][po : po + 64, 256 * i : 256 * (i + 1)], av
                )

        # ---------- Phase D: partial o-projection ----------
        with ExitStack() as dctx:
            wopool = dctx.enter_context(tc.tile_pool(name="wo", bufs=2 * NM))
            opool = dctx.enter_context(tc.tile_pool(name="o", bufs=3))
            wo_tiles = {}
            for cc in range(NM):
                for oh in range(2):
                    wt = wopool.tile([128, 512], F16, tag="wo", name=f"wo{cc}_{oh}")
                    nc.sync.dma_start(
                        out=wt,
                        in_=woT[128 * cc : 128 * (cc + 1), 512 * oh : 512 * (oh + 1)],
                    )
                    wo_tiles[(cc, oh)] = wt
            for st in range(NST):
                for oh in range(2):
                    ps = ps_mm.tile([128, 512], F32, tag="ps_mm")
                    for cc in range(NM):
                        nc.tensor.matmul(
                            ps,
                            _mm(attnoutT[cc][:, 128 * st : 128 * (st + 1)]),
                            _mm(wo_tiles[(cc, oh)]),
                            start=(cc == 0),
                            stop=(cc == NM - 1),
                        )
                    osb = opool.tile([128, 512], F32, tag="o")
                    nc.vector.tensor_copy(osb, ps)
                    nc.sync.dma_start(
                        out=out_part[
                            128 * st : 128 * (st + 1), 512 * oh : 512 * (oh + 1)
                        ],
                        in_=osb,
                    )


_NC_CACHE = None


def get_nc():
    global _NC_CACHE
    if _NC_CACHE is None:
        nc = bacc.Bacc(
            "TRN2", target_bir_lowering=False, debug=False, num_devices=NCORES
        )
        with tile.TileContext(nc) as tc:
            _build_kernel_body(tc)
        nc.compile()
        _NC_CACHE = nc
    return _NC_CACHE


def make_in_maps(x, Wq, bq, Wk, bk, Wv, bv, Wo, bo):
    """Per-core input dicts; core c handles batch c//2, head-group c%2."""
    in_maps = []
    for c in range(NCORES):
        b, hg = c // 2, c % 2
        sl = slice(C * hg, C * (hg + 1))
        in_maps.append(
            {
                "xT": np.ascontiguousarray(x[b].T),
                "wqT": np.ascontiguousarray(Wq[sl].T),
                "wkT": np.ascontiguousarray(Wk[sl].T),
                "wvT": np.ascontiguousarray(Wv[sl].T),
                "woT": np.ascontiguousarray(Wo[:, sl].T),
                "bq": np.ascontiguousarray(bq[sl]),
                "bk": np.ascontiguousarray(bk[sl]),
            }
        )
    return in_maps


def run_sharded(inputs, trace=False, trace_kwargs=None):
    """Compile (cached), run on 8 cores, return (results, BassKernelResults)."""
    from concourse.bass_utils import run_bass_kernel_spmd

    nc = get_nc()
    in_maps = make_in_maps(
        inputs["x"],
        inputs["Wq"],
        inputs["bq"],
        inputs["Wk"],
        inputs["bk"],
        inputs["Wv"],
        inputs["bv"],
        inputs["Wo"],
        inputs["bo"],
    )
    res = run_bass_kernel_spmd(
        nc,
        in_maps,
        core_ids=list(range(NCORES)),
        trace=trace,
        **(trace_kwargs or {}),
    )
    return res


def _assemble(inputs, results):
    x = inputs["x"]
    bv, Wo, bo = inputs["bv"], inputs["Wo"], inputs["bo"]
    # attn rows sum to 1, so the v-bias contributes bv @ Wo.T to every out row
    host_const = (
        bv.astype(np.float64) @ Wo.T.astype(np.float64) + bo.astype(np.float64)
    ).astype(np.float32)
    out = np.empty((B, S, H), np.float32)
    attn = np.empty((B, NH, S, S), np.float32)
    for c in range(NCORES):
        b, hg = c // 2, c % 2
        attn[b, 8 * hg : 8 * (hg + 1)] = results[c]["attn"]
    for b in range(B):
        out[b] = (
            results[2 * b]["out_part"] + results[2 * b + 1]["out_part"] + host_const
        )
    return out, attn


def kernel(**inputs):
    inputs = {k: np.asarray(v) for k, v in inputs.items()}
    res = run_sharded(inputs, trace=False)
    return _assemble(inputs, res.results)


# revision 8
# speedup vs baseline: 3.2977x; 3.2977x over previous
"""Trainium2 Bass kernel for DecoderAttention (B=4, S=2048, H=1024, NH=16).

Sharding: 8 cores = (batch b in 0..4) x (head-group hg in 0..2, 8 heads each).
Each core:
  - projects q/k/v for its 8 heads from x[b]      (weights pre-transposed on host)
  - causal attention, softmax without max-subtraction (scores bounded ~ +-3)
  - writes its [8, 2048, 2048] slice of `attn`
  - computes a partial o-projection (its 512 columns of the head dim)
Host sums the two partials per batch and adds the bias terms
(bo + bv @ Wo.T folds out of the device kernel because attn rows sum to 1).
"""

import numpy as np


def _import_concourse():
    try:
        import concourse.bass  # noqa: F401
    except ImportError:
        import sys

        for p in ("/root/.axon_site/_ro/trn_rl_repo", "/opt/trn_rl_repo"):
            if p not in sys.path:
                sys.path.append(p)


_import_concourse()

import concourse.bass as bass  # noqa: E402
import concourse.mybir as mybir  # noqa: E402
import concourse.tile as tile  # noqa: E402
from concourse import bacc  # noqa: E402
from concourse.masks import make_causal_mask, make_identity  # noqa: E402

B, S, H, NH = 4, 2048, 1024, 16
HD = H // NH  # 64
SCALE = float(HD) ** -0.5
NCORES = 8
C = H // 2  # 512 columns of the concat-head dim per core (8 heads)
F32 = mybir.dt.float32
F32R = mybir.dt.float32r
F16 = mybir.dt.float16
MASK_VAL = -1e10  # additive pre-softmax mask; exp(SCALE*(s+MASK_VAL)) == 0 in fp32

# Use float32r (full-rate fp32 matmul mode) for the big matmuls. The BIR
# verifier requires every fp32r-matmul operand to be *produced* as fp32r,
# so the feeding tiles (and their DRAM sources) are declared float32r.
MM_F32R = True
MM_DT = F32R if MM_F32R else F32


def _mm(ap):
    return ap


def _build_kernel_body(tc):
    nc = tc.nc
    from contextlib import ExitStack

    xT = nc.dram_tensor("xT", [H, S], MM_DT, kind="ExternalInput").ap()
    wqT = nc.dram_tensor("wqT", [H, C], MM_DT, kind="ExternalInput").ap()
    wkT = nc.dram_tensor("wkT", [H, C], MM_DT, kind="ExternalInput").ap()
    wvT = nc.dram_tensor("wvT", [H, C], MM_DT, kind="ExternalInput").ap()
    woT = nc.dram_tensor("woT", [C, H], F16, kind="ExternalInput").ap()
    bq = nc.dram_tensor("bq", [C], F32, kind="ExternalInput").ap()
    bk = nc.dram_tensor("bk", [C], F32, kind="ExternalInput").ap()
    attn = nc.dram_tensor("attn", [NH // 2, S, S], F32, kind="ExternalOutput").ap()
    out_part = nc.dram_tensor("out_part", [S, H], F32, kind="ExternalOutput").ap()

    HPC = NH // 2  # 8 heads per core
    NK = H // 128  # 8 contraction tiles for the projections
    NM = C // 128  # 4 o-tiles per projection
    NST = S // 128  # 16 s-tiles
    NSC = S // 512  # 4 s-chunks
    NQB = S // 128  # 16 query blocks

    with ExitStack() as ctx:
        consts = ctx.enter_context(tc.tile_pool(name="consts", bufs=1))
        identity_h = consts.tile([128, 128], F16)
        make_identity(nc, identity_h)
        cmask = consts.tile([128, 128], F32)
        make_causal_mask(nc, cmask, mask_val=MASK_VAL)

        qkpool = ctx.enter_context(tc.tile_pool(name="qk", bufs=2 * NM))
        vpool = ctx.enter_context(tc.tile_pool(name="v", bufs=NST))
        qT_sb = [qkpool.tile([128, S], MM_DT, tag="qk", name=f"qT{m}") for m in range(NM)]
        kT_sb = [qkpool.tile([128, S], MM_DT, tag="qk", name=f"kT{m}") for m in range(NM)]
        v_sb = [vpool.tile([128, C], F16, tag="v", name=f"v{s}") for s in range(NST)]

        ps_mm = ctx.enter_context(tc.tile_pool(name="ps_mm", bufs=2, space="PSUM"))

        # ---------- Phase B: q/k/v projections ----------
        with ExitStack() as bctx:
            xpool = bctx.enter_context(tc.tile_pool(name="x", bufs=NK))
            wtile = bctx.enter_context(tc.tile_pool(name="wt", bufs=2 * NK))
            wvpool = bctx.enter_context(tc.tile_pool(name="wv", bufs=NK))
            biasp = bctx.enter_context(tc.tile_pool(name="bias", bufs=2 * NM))

            xT_sb = [xpool.tile([128, S], MM_DT, tag="x", name=f"xT{k}") for k in range(NK)]
            for k in range(NK):
                nc.sync.dma_start(out=xT_sb[k], in_=xT[128 * k : 128 * (k + 1), :])

            bq_sb = [biasp.tile([128, 1], F32, tag="bias", name=f"bqs{m}") for m in range(NM)]
            bk_sb = [biasp.tile([128, 1], F32, tag="bias", name=f"bks{m}") for m in range(NM)]
            for m in range(NM):
                nc.sync.dma_start(
                    out=bq_sb[m], in_=bq[128 * m : 128 * (m + 1)].rearrange("(p one) -> p one", one=1)
                )
                nc.sync.dma_start(
                    out=bk_sb[m], in_=bk[128 * m : 128 * (m + 1)].rearrange("(p one) -> p one", one=1)
                )

            # qT / kT: out[o, s] accumulated over k; lhsT = w[k_tile, o_tile]
            for dst, wsrc, bias_sb in ((qT_sb, wqT, bq_sb), (kT_sb, wkT, bk_sb)):
                for m in range(NM):
                    wk_tiles = []
                    for k in range(NK):
                        wt = wtile.tile([128, 128], MM_DT, tag="wt", name=f"w{m}_{k}")
                        nc.sync.dma_start(
                            out=wt,
                            in_=wsrc[128 * k : 128 * (k + 1), 128 * m : 128 * (m + 1)],
                        )
                        wk_tiles.append(wt)
                    for sc in range(NSC):
                        ps = ps_mm.tile([128, 512], F32, tag="ps_mm")
                        for k in range(NK):
                            nc.tensor.matmul(
                                ps,
                                _mm(wk_tiles[k]),
                                _mm(xT_sb[k][:, 512 * sc : 512 * (sc + 1)]),
                                start=(k == 0),
                                stop=(k == NK - 1),
                            )
                        nc.scalar.activation(
                            dst[m][:, 512 * sc : 512 * (sc + 1)],
                            ps,
                            mybir.ActivationFunctionType.Identity,
                            bias=bias_sb[m],
                            scale=1.0,
                        )

            # v: out[s, c] (natural layout); lhsT = xT tile, rhs = wvT tile
            wv_tiles = []
            for k in range(NK):
                wt = wvpool.tile([128, C], MM_DT, tag="wv", name=f"wv{k}")
                nc.sync.dma_start(out=wt, in_=wvT[128 * k : 128 * (k + 1), :])
                wv_tiles.append(wt)
            for st in range(NST):
                ps = ps_mm.tile([128, 512], F32, tag="ps_mm")
                for k in range(NK):
                    nc.tensor.matmul(
                        ps,
                        _mm(xT_sb[k][:, 128 * st : 128 * (st + 1)]),
                        _mm(wv_tiles[k]),
                        start=(k == 0),
                        stop=(k == NK - 1),
                    )
                nc.vector.tensor_copy(v_sb[st], ps)

        # ---------- Phase C: attention ----------
        aopool = ctx.enter_context(tc.tile_pool(name="ao", bufs=NM))
        attnoutT = [aopool.tile([128, S], F16, tag="ao", name=f"ao{m}") for m in range(NM)]

        zpool = ctx.enter_context(tc.tile_pool(name="z", bufs=1))
        zero_t = zpool.tile([128, S], F32)
        nc.gpsimd.memset(zero_t, 0.0)

        ppool = ctx.enter_context(tc.tile_pool(name="p", bufs=4))
        phpool = ctx.enter_context(tc.tile_pool(name="ph", bufs=4))
        ptpool = ctx.enter_context(tc.tile_pool(name="pt", bufs=3))
        dpool = ctx.enter_context(tc.tile_pool(name="d", bufs=16))
        ps_pt = ctx.enter_context(tc.tile_pool(name="ps_pt", bufs=2, space="PSUM"))
        ps_av = ctx.enter_context(tc.tile_pool(name="ps_av", bufs=2, space="PSUM"))

        for h in range(HPC):
            mt = h // 2  # which 128-partition qT/kT tile
            po = 64 * (h % 2)  # partition offset of this head inside the tile
            qTh = qT_sb[mt][po : po + 64, :]
            kTh = kT_sb[mt][po : po + 64, :]
            for i in range(NQB // 2):
                p_pair = []
                kepair = 128 * (2 * i + 2)
                for qb in (2 * i, 2 * i + 1):
                    ke = 128 * (qb + 1)  # causal key extent for this q block
                    P = ppool.tile([128, S], F32, tag="p")
                    Ph = phpool.tile([128, S], F16, tag="ph")
                    p_pair.append(Ph)
                    nchunks = (ke + 511) // 512
                    dparts = []
                    for c in range(nchunks):
                        w = min(512, ke - 512 * c)
                        ps = ps_mm.tile([128, 512], F32, tag="ps_mm")
                        nc.tensor.matmul(
                            ps[:, :w],
                            _mm(qTh[:, 128 * qb : 128 * (qb + 1)]),
                            _mm(kTh[:, 512 * c : 512 * c + w]),
                            start=True,
                            stop=True,
                        )
                        if 512 * c <= ke - 128 < 512 * c + w:
                            off = (ke - 128) - 512 * c
                            nc.vector.tensor_add(
                                ps[:, off : off + 128], ps[:, off : off + 128], cmask
                            )
                        d = dpool.tile([128, 1], F32, tag="d")
                        nc.scalar.activation(
                            P[:, 512 * c : 512 * c + w],
                            ps[:, :w],
                            mybir.ActivationFunctionType.Exp,
                            scale=SCALE,
                            accum_out=d,
                        )
                        dparts.append(d)
                    dsum = dparts[0]
                    for d in dparts[1:]:
                        dn = dpool.tile([128, 1], F32, tag="d")
                        nc.vector.tensor_add(dn, dsum, d)
                        dsum = dn
                    invd = dpool.tile([128, 1], F32, tag="d")
                    nc.vector.reciprocal(invd, dsum)
                    # normalized fp16 shadow for the attn@v transposes; must
                    # trace BEFORE the in-place fp32 normalization below so it
                    # reads the raw exp output
                    nc.vector.tensor_scalar_mul(Ph[:, :ke], P[:, :ke], invd)
                    if qb % 2 == 0:
                        # the pair's attn@v runs to the odd block's extent
                        nc.gpsimd.memset(Ph[:, ke:kepair], 0.0)
                    nc.vector.tensor_scalar_mul(P[:, :ke], P[:, :ke], invd)
                    nc.sync.dma_start(
                        out=attn[h, 128 * qb : 128 * (qb + 1), :ke], in_=P[:, :ke]
                    )
                    if ke < S:
                        nc.sync.dma_start(
                            out=attn[h, 128 * qb : 128 * (qb + 1), ke:],
                            in_=zero_t[:, : S - ke],
                        )

                kt = 2 * i + 2  # key tiles covered by this pair
                av = ps_av.tile([64, 256], F32, tag="ps_av")
                for j in range(kt):
                    ptp = ps_pt.tile([128, 256], F16, tag="ps_pt")
                    nc.tensor.transpose(
                        ptp[:, 0:128], p_pair[0][:, 128 * j : 128 * (j + 1)], identity_h
                    )
                    nc.tensor.transpose(
                        ptp[:, 128:256], p_pair[1][:, 128 * j : 128 * (j + 1)], identity_h
                    )
                    pt = ptpool.tile([128, 256], F16, tag="pt")
                    if j % 2 == 0:
                        nc.scalar.copy(pt, ptp)
                    else:
                        nc.vector.tensor_copy(pt, ptp)
                    nc.tensor.matmul(
                        av,
                        _mm(v_sb[j][:, 64 * h : 64 * h + 64]),
                        _mm(pt),
                        start=(j == 0),
                        stop=(j == kt - 1),
                    )
                nc.scalar.copy(
                    attnoutT[mt][po : po + 64, 256 * i : 256 * (i + 1)], av
                )

        # ---------- Phase D: partial o-projection ----------
        with ExitStack() as dctx:
            wopool = dctx.enter_context(tc.tile_pool(name="wo", bufs=2 * NM))
            opool = dctx.enter_context(tc.tile_pool(name="o", bufs=3))
            wo_tiles = {}
            for cc in range(NM):
                for oh in range(2):
                    wt = wopool.tile([128, 512], F16, tag="wo", name=f"wo{cc}_{oh}")
                    nc.sync.dma_start(
                        out=wt,
                        in_=woT[128 * cc : 128 * (cc + 1), 512 * oh : 512 * (oh + 1)],
                    )
                    wo_tiles[(cc, oh)] = wt
            for st in range(NST):
                for oh in range(2):
                    ps = ps_mm.tile([128, 512], F32, tag="ps_mm")
                    for cc in range(NM):
                        nc.tensor.matmul(
                            ps,
                            _mm(attnoutT[cc][:, 128 * st : 128 * (st + 1)]),
                            _mm(wo_tiles[(cc, oh)]),
                            start=(cc == 0),
                            stop=(cc == NM - 1),
                        )
                    osb = opool.tile([128, 512], F32, tag="o")
                    nc.vector.tensor_copy(osb, ps)
                    nc.sync.dma_start(
                        out=out_part[
                            128 * st : 128 * (st + 1), 512 * oh : 512 * (oh + 1)
                        ],
                        in_=osb,
                    )


_NC_CACHE = None


def get_nc():
    global _NC_CACHE
    if _NC_CACHE is None:
        nc = bacc.Bacc(
            "TRN2", target_bir_lowering=False, debug=False, num_devices=NCORES
        )
        with tile.TileContext(nc) as tc:
            _build_kernel_body(tc)
        nc.compile()
        _NC_CACHE = nc
    return _NC_CACHE


def make_in_maps(x, Wq, bq, Wk, bk, Wv, bv, Wo, bo):
    """Per-core input dicts; core c handles batch c//2, head-group c%2."""
    in_maps = []
    for c in range(NCORES):
        b, hg = c // 2, c % 2
        sl = slice(C * hg, C * (hg + 1))
        in_maps.append(
            {
                "xT": np.ascontiguousarray(x[b].T),
                "wqT": np.ascontiguousarray(Wq[sl].T),
                "wkT": np.ascontiguousarray(Wk[sl].T),
                "wvT": np.ascontiguousarray(Wv[sl].T),
                "woT": np.ascontiguousarray(Wo[:, sl].T),
                "bq": np.ascontiguousarray(bq[sl]),
                "bk": np.ascontiguousarray(bk[sl]),
            }
        )
    return in_maps


def run_sharded(inputs, trace=False, trace_kwargs=None):
    """Compile (cached), run on 8 cores, return (results, BassKernelResults)."""
    from concourse.bass_utils import run_bass_kernel_spmd

    nc = get_nc()
    in_maps = make_in_maps(
        inputs["x"],
        inputs["Wq"],
        inputs["bq"],
        inputs["Wk"],
        inputs["bk"],
        inputs["Wv"],
        inputs["bv"],
        inputs["Wo"],
        inputs["bo"],
    )
    res = run_bass_kernel_spmd(
        nc,
        in_maps,
        core_ids=list(range(NCORES)),
        trace=trace,
        **(trace_kwargs or {}),
    )
    return res


def _assemble(inputs, results):
    x = inputs["x"]
    bv, Wo, bo = inputs["bv"], inputs["Wo"], inputs["bo"]
    # attn rows sum to 1, so the v-bias contributes bv @ Wo.T to every out row
    host_const = (
        bv.astype(np.float64) @ Wo.T.astype(np.float64) + bo.astype(np.float64)
    ).astype(np.float32)
    out = np.empty((B, S, H), np.float32)
    attn = np.empty((B, NH, S, S), np.float32)
    for c in range(NCORES):
        b, hg = c // 2, c % 2
        attn[b, 8 * hg : 8 * (hg + 1)] = results[c]["attn"]
    for b in range(B):
        out[b] = (
            results[2 * b]["out_part"] + results[2 * b + 1]["out_part"] + host_const
        )
    return out, attn


def kernel(**inputs):
    inputs = {k: np.asarray(v) for k, v in inputs.items()}
    res = run_sharded(inputs, trace=False)
    return _assemble(inputs, res.results)
